# revision 67
# baseline (speedup 1.0000x reference)
"""DenseGAT layer (LN -> masked 12-head attention -> LN -> GELU FFN, residuals)
on 8 Trainium2 NeuronCores, data-parallel over the batch (4 graphs/core).

Math notes (validated against the reference in fp64/numpy):
- LN affine (g,b) is folded into the following projection weights on the host,
  along with the 1/sqrt(hd) attention scale (into wq) and 0-biases.
- Scores are computed directly transposed, scoresT[m,n] = kT_h^T-slice @ qT_h,
  so softmax's reduction lands on the matmul contraction axis: masked exp
  weights eT are multiplied by the host-transposed 0/1 mask, and the
  denominator comes for free as a 65th "ones" column of V. No max-subtraction
  is needed: |scores| < ~3 for this model family (exp is safe in fp32).
- The projection/FFN matmuls (QKV, O, FFN1, FFN2) run in fp8e4m3 with
  DoubleRow perf mode (2 k-tiles of 128 contracted per pass, ~1.5-2x PE
  throughput); operands carry power-of-2 scales (z x16, weights x64/x512,
  ao x32 via a 1/32 "ones" column) that are divided back out at each PSUM
  evacuation. Empirical max-rel-err of this scheme vs the fp32 reference is
  ~1.8e-2 (gate 2e-2, deterministic inputs). Scores/AV matmuls stay bf16
  (contraction 64 gains nothing from DoubleRow); LN statistics and softmax
  normalization stay fp32; x/out/outres are fp16.
"""
import numpy as np
import ml_dtypes

import concourse.bass as bass
import concourse.mybir as mybir
import concourse.tile as tile
from concourse import masks
from concourse.bass_utils import run_bass_kernel_spmd

bf16 = ml_dtypes.bfloat16
f8e4 = ml_dtypes.float8_e4m3    # TRN fp8_exp4: max normal +-240
FP32 = mybir.dt.float32
FP16 = mybir.dt.float16
BF16 = mybir.dt.bfloat16
F8E4 = mybir.dt.float8e4
F = mybir.ActivationFunctionType
OP = mybir.AluOpType
DR = mybir.MatmulPerfMode.DoubleRow

B, N, D, H = 32, 512, 768, 12
HD = D // H            # 64
NCORES = 8
GPC = B // NCORES      # graphs per core
NT = N // 128          # 4 n-tiles (also m-chunks)
DC = D // 128          # 6 d-chunks
DP = DC // 2           # 3 d-chunk PAIRS (DoubleRow contraction steps)
FC = 4 * D // 128      # 24 ffn chunks
FP = FC // 2           # 12 ffn chunk pairs
EPS = 1e-5

# fp8 operand scales (powers of 2; divided back out at PSUM evacuation)
SZ = 16.0              # z / z2 (LN outputs, unit variance)
SWQ = 512.0            # wq (carries the extra 1/8 attention scale)
SW = 64.0              # wk, wv, wo, w1, w2 (~0.02 scale weights)
SAO = 32.0             # ao (attention out, via 1/SAO ones column in V)


def _split_waits(nc):
    """This walrus accepts one sync-wait per instruction (two for
    EventSemaphore); hoist excess waits onto same-engine nops inserted before
    the instruction (engines dispatch in order, so semantics are preserved)."""
    counter = 0
    for f in nc.m.functions:
        for blk in f.blocks:
            insts = blk.instructions
            i = 0
            while i < len(insts):
                inst = insts[i]
                si = getattr(inst, "sync_info", None)
                waits = list(si.on_wait) if si is not None and si.on_wait else []
                cap = 2 if isinstance(inst, mybir.InstEventSemaphore) else 1
                if len(waits) > cap:
                    si.on_wait = waits[-cap:]
                    for c in waits[:-cap]:
                        counter += 1
                        nop = mybir.InstNoOp(name=f"wsplit-{counter}", ins=[], outs=[])
                        nop.engine = inst.engine
                        nop.sync_info = type(si)(on_wait=[c], on_update=[])
                        insts.insert(i, nop)
                        i += 1
                i += 1


# ALL inputs (weights, biases, per-core mask, and x itself) live in ONE 1-D
# bf16 blob: every PJRT argument costs measurable per-exec dispatch time
# through the tunnel, so the kernel takes a single input tensor. fp8 weights
# are packed two-bytes-per-bf16-element (x4 fp16 likewise) and bitcast on
# chip.
_SPEC = [
    # (name, element count in the STORED dtype, is_fp8_packed)
    ("wq", D * D // 2, True), ("wk", D * D // 2, True),
    ("wv", D * D // 2, True), ("wo", D * D // 2, True),
    ("w1", FC * 128 * D // 2, True), ("w2", 4 * D * D // 2, True),
    ("bv", D, False), ("bo", D, False), ("b2f", D, False),
    ("bq", D, False), ("bk", D, False), ("b1f", 4 * D, False),
    ("ident", 128 * 128, False),     # PE-transpose identity (gpsimd
                                     # affine_select is slow on HW)
    ("maskT4", GPC * N * N, False),  # per-core entries last
    ("x4", GPC * N * D, False),      # fp16 bytes in bf16 slots (bitcast)
]
_BF16_OFF = {}
_off = 0
for _nm, _sz, _f8 in _SPEC:
    _BF16_OFF[_nm] = _off
    _off += _sz
_BF16_TOT = _off


def _build_module():
    nc = bass.Bass()
    dram = {
        "wb": nc.dram_tensor("wb", [_BF16_TOT], BF16, kind="ExternalInput"),
    }
    out4 = nc.dram_tensor("out4", [GPC, N, D], FP16, kind="ExternalOutput")

    with tile.TileContext(nc) as tc:
        _emit(nc, tc, dram, out4)
    _split_waits(nc)
    return nc


def _wb(dram, name, sz, extra_off=0):
    return dram["wb"][_BF16_OFF[name] + extra_off:
                      _BF16_OFF[name] + extra_off + sz]


def _emit(nc, tc, dram, out4):
    import contextlib
    ctx = contextlib.ExitStack()
    with ctx:
        res = ctx.enter_context(tc.tile_pool(name="res", bufs=1))       # resident
        xp = ctx.enter_context(tc.tile_pool(name="xp", bufs=2))
        mkp = ctx.enter_context(tc.tile_pool(name="mkp", bufs=1))
        znat = ctx.enter_context(tc.tile_pool(name="znat", bufs=2))     # z / ao / z2
        ztp = ctx.enter_context(tc.tile_pool(name="ztp", bufs=2))       # zT / aoT / z2T (bf16 staging)
        zt8p = ctx.enter_context(tc.tile_pool(name="zt8p", bufs=2))     # fp8 transposed acts
        qkp = ctx.enter_context(tc.tile_pool(name="qkp", bufs=1))       # qT, kT
        vp = ctx.enter_context(tc.tile_pool(name="vp", bufs=1))
        etp = ctx.enter_context(tc.tile_pool(name="etp", bufs=7))
        gp = ctx.enter_context(tc.tile_pool(name="gp", bufs=1))         # gT resident per graph
        orp = ctx.enter_context(tc.tile_pool(name="orp", bufs=1))       # outres
        fin = ctx.enter_context(tc.tile_pool(name="fin", bufs=2))
        sm = ctx.enter_context(tc.tile_pool(name="sm", bufs=4))         # small stats tiles
        mm = ctx.enter_context(tc.tile_pool(name="mm", bufs=5, space="PSUM"))
        av = ctx.enter_context(tc.tile_pool(name="av", bufs=3, space="PSUM"))

        # ---- input prefetch (graph 0 first so weight loads don't delay it) --
        x_tiles, mk_tiles = {}, {}
        zT8s, qTs, kTs, vsbs = {}, {}, {}, {}

        def xsrc(b):
            return _wb(dram, "x4", N * D, extra_off=b * N * D).bitcast(
                FP16).rearrange("(t p d) -> p t d", p=128, d=D)

        def load_x(b):
            x_tiles[b] = xp.tile([128, NT, D], FP16, name=f"x{b}", tag="x")
            nc.sync.dma_start(out=x_tiles[b], in_=xsrc(b))

        def load_mask(b):
            mk_tiles[b] = mkp.tile([128, NT, N], BF16, name=f"mk{b}", tag="mk")
            nc.sync.dma_start(
                out=mk_tiles[b],
                in_=_wb(dram, "maskT4", N * N, extra_off=b * N * N)
                .rearrange("(c p n) -> p c n", p=128, n=N))

        # ---- one-time loads -------------------------------------------------
        def load_w8(name, chunks, cols8):
            """fp8 weight packed as bf16 byte pairs: load bf16 [128, chunks,
            cols8/2], return the fp8 [128, chunks, cols8] bitcast view."""
            t = res.tile([128, chunks, cols8 // 2], BF16, tag=name)
            nc.sync.dma_start(
                out=t, in_=_wb(dram, name, chunks * 128 * cols8 // 2)
                .rearrange("(c p d) -> p c d", p=128, d=cols8 // 2))
            return t.bitcast(F8E4)

        # startup: the serial DMA pipe is the bottleneck, so order it by need:
        # all 4 x tiles, then wq's first column-block (enough for qk j=0..2),
        # then the rest. Per-tile LN runs as each x tile lands; graph 0's zT
        # rides the PE (idle at startup) straight into fp8 via the ACT evac.
        eps_t = res.tile([128, 1], FP32, tag="eps")
        nc.vector.memset(eps_t, EPS)
        x_tiles[0] = xp.tile([128, NT, D], FP16, name="x0", tag="x")
        x0src = xsrc(0)
        z0_sb = znat.tile([128, NT, D], BF16, tag="znat", name="z0")
        zT0 = zt8p.tile([128, DC, N], F8E4, tag="zt8", name="zT0")
        for t_ in range(NT):
            nc.sync.dma_start(out=x_tiles[0][:, t_, :], in_=x0src[:, t_, :])
        # identity right behind x0 (32KB): the first zT0 transpose needs it
        ident = res.tile([128, 128], BF16, tag="ident")
        nc.sync.dma_start(out=ident, in_=_wb(dram, "ident", 128 * 128)
                          .rearrange("(p d) -> p d", p=128))
        # wq first column-block first (fp8 cols j<3 of every dc chunk, enough
        # for qk j=0..2), then the rest after the LN chain is queued
        wq_t = res.tile([128, DC, D // 2], BF16, tag="wq")
        wq_src = _wb(dram, "wq", DC * 128 * D // 2).rearrange(
            "(c p d) -> p c d", p=128, d=D // 2)
        nc.sync.dma_start(out=wq_t[:, :, 0:64], in_=wq_src[:, :, 0:64])
        nc.sync.dma_start(out=wq_t[:, :, 64:192], in_=wq_src[:, :, 64:192])
        wq8 = wq_t.bitcast(F8E4)
        # graph 0's zT via PE transposes: the PE is idle at startup while the
        # serial DMA pipe (x, wq, wk...) is the scarce resource. The ACT evac
        # applies the fp8 z-scale (x16) for free.
        for t_ in range(NT):
            _layernorm_tile(nc, sm, x_tiles[0], z0_sb, eps_t, t_)
            for dcb in range(DC):
                pst = mm.tile([128, 128], BF16, tag="mm")
                nc.tensor.matmul(pst[:, :],
                                 z0_sb[:, t_, dcb * 128:(dcb + 1) * 128],
                                 ident[:, :], is_transpose=True)
                nc.scalar.activation(
                    out=zT0[:, dcb, t_ * 128:(t_ + 1) * 128], in_=pst[:, :],
                    func=F.Identity, scale=SZ)
        zT8s[0] = zT0
        nc.sync.dma_start(out=wq_t[:, :, 192:384], in_=wq_src[:, :, 192:384])
        wk8 = load_w8("wk", DC, D)
        wv8 = load_w8("wv", DC, D)
        load_mask(0)

        bq_sb = res.tile([128, DC], FP32, tag="bq")
        bq_lo = res.tile([128, DC], BF16, tag="bqlo")
        nc.sync.dma_start(out=bq_lo,
                          in_=_wb(dram, "bq", D).rearrange("(c p) -> p c", p=128))
        nc.scalar.activation(out=bq_sb, in_=bq_lo, func=F.Identity)
        bk_sb = res.tile([128, DC], FP32, tag="bk")
        bk_lo = res.tile([128, DC], BF16, tag="bklo")
        nc.sync.dma_start(out=bk_lo,
                          in_=_wb(dram, "bk", D).rearrange("(c p) -> p c", p=128))
        nc.scalar.activation(out=bk_sb, in_=bk_lo, func=F.Identity)
        b1f_sb = res.tile([128, FC], BF16, tag="b1f")
        nc.sync.dma_start(out=b1f_sb,
                          in_=_wb(dram, "b1f", 4 * D).rearrange("(c p) -> p c", p=128))

        def bcast(name):
            t = res.tile([128, D], BF16, tag=name + "b")
            src = _wb(dram, name, D)
            nc.sync.dma_start(out=t, in_=bass.AP(
                tensor=src.tensor, offset=src.offset, ap=[[0, 128]] + list(src.ap)))
            return t

        bv_b = bcast("bv")
        bo_b = bcast("bo")      # host-scaled by SAO*SW (O-proj psum scale)
        b2_b = bcast("b2f")     # host-scaled by SW (FFN2 psum scale)
        # all-ones [1, 128] stationary: a K=1 matmul appended to a PSUM
        # accumulation group adds ones^T @ bias_row = the bias broadcast over
        # all 128 output partitions, for free on the PE
        ones1 = res.tile([1, 128], BF16, tag="ones1")
        nc.vector.memset(ones1, 1.0)

        # ---- graph-level software pipeline ---------------------------------
        # LN1/zT and QKV for graph b+1 are emitted inside graph b's body so
        # each engine's static instruction order interleaves next-graph work
        # into this graph's stall windows.

        zTbs = {}

        def ln_z_zT(b):
            z_sb = znat.tile([128, NT, D], BF16, tag="znat", name=f"z{b}")
            zTb = ztp.tile([128, DC, N], BF16, tag="zt", name=f"zTb{b}")
            zT8 = zt8p.tile([128, DC, N], F8E4, tag="zt8", name=f"zT8{b}")
            _layernorm(nc, sm, x_tiles[b], z_sb, eps_t)
            for t in range(NT):
                nc.sync.dma_start_transpose(
                    out=zTb[:, :, t * 128:(t + 1) * 128], in_=z_sb[:, t, :])
            zTbs[b] = zTb
            zT8s[b] = zT8

        def zT_convert(b):
            # emitted a few attention pairs after ln_z_zT so the transposes
            # are long done and the converts neither head-of-line-block the
            # DVE FIFO nor finish after the next graph's qk_proj needs them
            zTb, zT8 = zTbs.pop(b), zT8s[b]
            for t in range(NT):
                nc.vector.tensor_scalar(
                    out=zT8[:, :, t * 128:(t + 1) * 128],
                    in0=zTb[:, :, t * 128:(t + 1) * 128],
                    scalar1=SZ, scalar2=None, op0=OP.mult)

        def qk_proj(b, split_n=False):
            zT8 = zT8s[b]
            qT = qkp.tile([128, DC, N], BF16, tag="q", name=f"qT{b}")
            kT = qkp.tile([128, DC, N], BF16, tag="k", name=f"kT{b}")
            nsplits = ((0, 256), (256, 512)) if split_n else ((0, 512),)
            for wsb, bsb, inv, outT, on_act in (
                    (wq8, bq_sb, 1.0 / (SZ * SWQ), qT, True),
                    (wk8, bk_sb, 1.0 / (SZ * SW), kT, True)):
                for j in range(DC):
                    ps = mm.tile([128, N], FP32, tag="mm")
                    for n0, n1 in nsplits:
                        for dp in range(DP):
                            nc.tensor.matmul(
                                ps[:, n0:n1],
                                wsb[:, 2 * dp:2 * dp + 2, j * 128:(j + 1) * 128],
                                zT8[:, 2 * dp:2 * dp + 2, n0:n1],
                                start=(dp == 0), stop=(dp == DP - 1),
                                perf_mode=DR)
                    # evacs split across ACT (q) and DVE (k) to balance the
                    # two engines' loads in the post-attention window
                    if on_act:
                        nc.scalar.activation(
                            out=outT[:, j, :], in_=ps[:, :], func=F.Identity,
                            scale=inv, bias=bsb[:, j:j + 1])
                    else:
                        nc.vector.tensor_scalar(
                            out=outT[:, j, :], in0=ps[:, :],
                            scalar1=inv, scalar2=bsb[:, j:j + 1],
                            op0=OP.mult, op1=OP.add)
            qTs[b], kTs[b] = qT, kT

        def v_proj(b):
            zT8 = zT8s[b]
            v_sb = vp.tile([128, NT, H, HD + 1], BF16, name=f"v{b}", tag="v")
            for mc in range(NT):
                for half in range(2):
                    # psum from the av pool: it is idle in this window (pav
                    # tiles are dead post-attention) while the mm ring's WAR
                    # against the O-proj evacs would stall these matmuls
                    ps = av.tile([128, 384], FP32, tag="av")
                    for dp in range(DP):
                        nc.tensor.matmul(
                            ps[:, :],
                            zT8[:, 2 * dp:2 * dp + 2, mc * 128:(mc + 1) * 128],
                            wv8[:, 2 * dp:2 * dp + 2, half * 384:(half + 1) * 384],
                            start=(dp == 0), stop=(dp == DP - 1),
                            perf_mode=DR)
                    nc.vector.scalar_tensor_tensor(
                        out=v_sb[:, mc, half * 6:(half + 1) * 6, 0:HD],
                        in0=ps.rearrange("p (h d) -> p h d", d=HD),
                        scalar=1.0 / (SZ * SW),
                        in1=bv_b[:, half * 384:(half + 1) * 384].rearrange(
                            "p (h d) -> p h d", d=HD),
                        op0=OP.mult, op1=OP.add)
            # ones column carries 1/SAO so the evac reciprocal bakes the fp8
            # ao scale into the normalized output for free
            nc.vector.memset(v_sb[:, :, :, HD:HD + 1], 1.0 / SAO)
            vsbs[b] = v_sb

        eTs_pre = {}

        def scores_pair(p, qT, kT, mk_sb, gtag):
            eT = etp.tile([128, NT, 2, N], BF16, tag="et", name=f"eT{gtag}_{p}")
            for mc in range(NT):
                for sub in range(2):
                    off = sub * 64
                    ps = mm.tile([128, N], FP32, tag="mm")
                    nc.tensor.matmul(
                        ps[:, :],
                        kT[off:off + 64, p, mc * 128:(mc + 1) * 128],
                        qT[off:off + 64, p, :],
                        start=True, stop=True)
                    nc.scalar.activation(out=eT[:, mc, sub, :], in_=ps[:, :],
                                         func=F.Exp)
                # one masked-multiply for both sub-heads (mask broadcast over
                # the sub dim) — halves the DVE op count in this hot window
                mk1 = mk_sb[:, mc, :]
                mkb = bass.AP(tensor=mk1.tensor, offset=mk1.offset,
                              ap=[mk1.ap[0], [0, 2], mk1.ap[1]])
                nc.vector.tensor_tensor(out=eT[:, mc, :, :],
                                        in0=eT[:, mc, :, :],
                                        in1=mkb, op=OP.mult)
            return eT

        qk_proj(0, split_n=True)
        v_proj(0)
        # x1 isn't needed until LN1(1) during graph 0's attention (~30us);
        # keep its transfer out of the startup pipe
        with tc.tile_wait_until(0.012):
            load_x(1)
        # wo/w1/w2 aren't needed until O-proj/FFN of graph 0 (~60-90us in);
        # keep their transfers out of the startup DMA-pipe rush
        with tc.tile_wait_until(0.018):
            wo8 = load_w8("wo", DC, D)
            w2_8 = load_w8("w2", FC, D)
        with tc.tile_wait_until(0.025):
            w1t = res.tile([128, FC, D // 2], BF16, tag="w1")
            nc.gpsimd.dma_start(
                out=w1t, in_=_wb(dram, "w1", FC * 128 * D // 2)
                .rearrange("(f p d) -> p f d", p=128, d=D // 2))
            w1_8 = w1t.bitcast(F8E4)
        w1v = w1_8.rearrange("p f (c m) -> p f c m", m=128)

        for b in range(GPC):
            x_sb = x_tiles[b]
            mk_sb = mk_tiles.pop(b)
            qT, kT, v_sb = qTs.pop(b), kTs.pop(b), vsbs.pop(b)

            # ---- attention: 6 head-pairs, software-pipelined ----
            ao = znat.tile([128, NT, D], BF16, tag="znat", name=f"ao{b}")
            pav = {}

            def scores_block(p):
                return scores_pair(p, qT, kT, mk_sb, b)

            def av_block(p, eT):
                half = p // 3
                if half not in pav:
                    pav[half] = [av.tile([128, 6 * (HD + 1)], FP32, tag="av",
                                         name=f"pav{half}_{nc4i}")
                                 for nc4i in range(NT)]
                for sub in range(2):
                    h = 2 * p + sub
                    lane = h - half * 6
                    for nc4 in range(NT):
                        for mc in range(NT):
                            nc.tensor.matmul(
                                pav[half][nc4][:, lane * 65:lane * 65 + 65],
                                eT[:, mc, sub, nc4 * 128:(nc4 + 1) * 128],
                                v_sb[:, mc, h, :],
                                start=(mc == 0), stop=(mc == NT - 1))

            def evac_block(half):
                for nc4 in range(NT):
                    rec = sm.tile([128, 6], FP32, tag="rec")
                    nc.vector.reciprocal(out=rec, in_=pav[half][nc4][:, HD::HD + 1])
                    rb = bass.AP(tensor=rec.tensor, offset=rec.offset,
                                 ap=[rec.ap[0], [rec.ap[1][0], 6], [0, HD]])
                    nc.vector.tensor_tensor(
                        out=ao[:, nc4, half * 384:(half + 1) * 384].rearrange(
                            "p (h d) -> p h d", d=HD),
                        in0=pav[half][nc4].rearrange(
                            "p (h s) -> p h s", s=HD + 1)[:, :, 0:HD],
                        in1=rb, op=OP.mult)

            pre = eTs_pre.pop(b, None)
            if pre:
                queue = list(pre)
                nxt = len(pre)
            else:
                queue = [scores_block(0)]
                nxt = 1
            for p in range(6):
                if nxt < 6:
                    queue.append(scores_block(nxt))
                    nxt += 1
                eT_cur = queue.pop(0)
                av_block(p, eT_cur)
                if p == 0 and b + 1 < GPC:
                    # next graph's LN1+transpose: its DVE/ACT/DMA work fills
                    # this graph's attention slack, and zT(b+1) is ready before
                    # the post-attention QK(b+1) matmuls need it.
                    ln_z_zT(b + 1)
                if p == 3 and b + 1 < GPC:
                    zT_convert(b + 1)
                if p == 2:
                    evac_block(0)
                elif p == 5:
                    evac_block(1)
                    if b + 1 < GPC:
                        load_mask(b + 1)

            # aoT (bf16 staging transpose, per-tile fp8 convert rides behind)
            aoTb = ztp.tile([128, DC, N], BF16, tag="zt", name=f"aoTb{b}")
            aoT8 = zt8p.tile([128, DC, N], F8E4, tag="zt8", name=f"aoT8{b}")
            for t in range(NT):
                nc.sync.dma_start_transpose(out=aoTb[:, :, t * 128:(t + 1) * 128],
                                            in_=ao[:, t, :])
                nc.vector.tensor_scalar(
                    out=aoT8[:, :, t * 128:(t + 1) * 128],
                    in0=aoTb[:, :, t * 128:(t + 1) * 128],
                    scalar1=1.0, scalar2=None, op0=OP.mult)

            # next graph's q/k projections fill the aoT-transpose latency
            if b + 1 < GPC:
                qk_proj(b + 1)

            # O projection + residual -> outres (fp16), LN2 interleaved per
            # tile so z2T(t) transposes overlap O-proj of tiles t+1..3 (the
            # DVE/ACT LN2 work for tile t queues before tile t+1's evac).
            outres = orp.tile([128, NT, D], FP16, name=f"or{b}", tag="or")
            z2 = znat.tile([128, NT, D], BF16, tag="znat", name=f"z2{b}")
            z2Tb = ztp.tile([128, DC, N], BF16, tag="zt", name=f"z2Tb{b}")
            z2T8 = zt8p.tile([128, DC, N], F8E4, tag="zt8", name=f"z2T8{b}")
            for nc4 in range(NT):
                for half in range(2):
                    ps = mm.tile([128, 384], FP32, tag="mm")
                    for dp in range(DP):
                        nc.tensor.matmul(
                            ps[:, :],
                            aoT8[:, 2 * dp:2 * dp + 2, nc4 * 128:(nc4 + 1) * 128],
                            wo8[:, 2 * dp:2 * dp + 2, half * 384:(half + 1) * 384],
                            start=(dp == 0), stop=False,
                            perf_mode=DR)
                    # + bo (host-scaled) via a K=1 ones-row matmul in-group
                    nc.tensor.matmul(
                        ps[:, :], ones1[:, :],
                        bo_b[0:1, half * 384:(half + 1) * 384],
                        start=False, stop=True)
                    # evacuation (descale) + residual
                    nc.vector.scalar_tensor_tensor(
                        out=outres[:, nc4, half * 384:(half + 1) * 384],
                        in0=ps[:, :], scalar=1.0 / (SAO * SW),
                        in1=x_sb[:, nc4, half * 384:(half + 1) * 384],
                        op0=OP.mult, op1=OP.add)
                # LN2(tile nc4) via DVE bn_stats (keeps ACT free for the
                # next-graph exp precompute below)
                t = nc4
                _layernorm_tile(nc, sm, outres, z2, eps_t, t)
                # ACT queue: fires right after this tile's rstd instead of
                # queuing behind unrelated SP configs
                nc.scalar.dma_start_transpose(
                    out=z2Tb[:, :, t * 128:(t + 1) * 128], in_=z2[:, t, :])

            # next graph's v-proj + first scores pair: their PE/ACT work fills
            # the LN2 -> z2T transpose -> fp8 convert latency before FFN1.
            # The z2T8 converts are emitted after v_proj so the DVE FIFO
            # reaches the O-proj evacs (v_proj's mm-ring WAR blocker) sooner.
            if b + 1 < GPC:
                v_proj(b + 1)
            for t in range(NT):
                nc.vector.tensor_scalar(
                    out=z2T8[:, :, t * 128:(t + 1) * 128],
                    in0=z2Tb[:, :, t * 128:(t + 1) * 128],
                    scalar1=SZ, scalar2=None, op0=OP.mult)
            if b + 1 < GPC:
                eTs_pre.setdefault(b + 1, []).append(
                    scores_pair(0, qTs[b + 1], kTs[b + 1], mk_tiles[b + 1],
                                b + 1))

            # FFN1 + gelu -> gT (fp8). First chunks run as n-splits so they
            # only need the early z2T transposes (later ones still in flight
            # after O-proj).
            gT8 = gp.tile([128, FC, N], F8E4, name=f"gT{b}", tag="gT")
            for fc in range(FC):
                ps = mm.tile([128, N], FP32, tag="mm")
                if fc < 2:        # quarters: starts after z2T transpose t=0
                    nsp = ((0, 128), (128, 256), (256, 384), (384, 512))
                elif fc < 4:      # halves: needs t=0,1
                    nsp = ((0, 256), (256, 512))
                else:
                    nsp = ((0, 512),)
                for n0, n1 in nsp:
                    for dp in range(DP):
                        nc.tensor.matmul(
                            ps[:, n0:n1],
                            w1v[:, fc, 2 * dp:2 * dp + 2, :],
                            z2T8[:, 2 * dp:2 * dp + 2, n0:n1],
                            start=(dp == 0), stop=(dp == DP - 1),
                            perf_mode=DR)
                # NOTE: no exp ops may interleave here — gelu and exp live in
                # different ACT table sets, and each switch costs a ~2.7us
                # ACT_TABLE_LOAD (invisible to TimelineSim)
                nc.scalar.activation(out=gT8[:, fc, :], in_=ps[:, :], func=F.Gelu,
                                     bias=b1f_sb[:, fc:fc + 1],
                                     scale=1.0 / (SZ * SW))

            if b + 2 < GPC:
                load_x(b + 2)

            # FFN2 + final residual -> out. ACT is idle here, so pre-compute
            # two more of the next graph's score pairs (exp on ACT) — the
            # next attention window then starts with 4/6 pairs done.
            for nc4 in range(NT):
                for half in range(2):
                    ps = mm.tile([128, 384], FP32, tag="mm")
                    for fp in range(FP):
                        nc.tensor.matmul(
                            ps[:, :],
                            gT8[:, 2 * fp:2 * fp + 2, nc4 * 128:(nc4 + 1) * 128],
                            w2_8[:, 2 * fp:2 * fp + 2, half * 384:(half + 1) * 384],
                            start=(fp == 0), stop=False,
                            perf_mode=DR)
                    # + b2f (host-scaled) in-group, as with bo above
                    nc.tensor.matmul(
                        ps[:, :], ones1[:, :],
                        b2_b[0:1, half * 384:(half + 1) * 384],
                        start=False, stop=True)
                    ft = fin.tile([128, 384], FP16, tag="fin")
                    nc.vector.scalar_tensor_tensor(
                        out=ft, in0=ps[:, :], scalar=1.0 / SW,
                        in1=outres[:, nc4, half * 384:(half + 1) * 384],
                        op0=OP.mult, op1=OP.add)
                    nc.sync.dma_start(
                        out=out4[b, nc4 * 128:(nc4 + 1) * 128,
                                 half * 384:(half + 1) * 384],
                        in_=ft)
                # pre-compute next graph's score pairs here: the FFN2 window
                # has no ACT work of its own, and exp shares the loaded ACT
                # table set (the gelu run above already paid its two loads)
                if b + 1 < GPC and nc4 < 3:
                    eTs_pre[b + 1].append(
                        scores_pair(nc4 + 1, qTs[b + 1], kTs[b + 1],
                                    mk_tiles[b + 1], b + 1))
            del x_tiles[b]


def _layernorm_tile(nc, sm, src, dst, eps_t, t):
    """Single-tile LN chain (stats -> rstd -> z) for pipeline fill: no
    cross-tile batching, so tile t's output is ready as soon as tile t's
    input is."""
    stats = sm.tile([128, 3, 6], FP32, tag="bn")
    xg = src[:, t, :].rearrange("p (s d) -> p s d", s=3)
    for s in range(3):
        nc.vector.bn_stats(out=stats[:, s, :], in_=xg[:, s, :])
    mv = sm.tile([128, 2], FP32, tag="mv")
    nc.vector.bn_aggr(out=mv, in_=stats)
    lnv = sm.tile([128, 1], FP32, tag="lnv")
    nc.scalar.activation(out=lnv, in_=mv[:, 1:2], func=F.Ln, bias=eps_t)
    rstd = sm.tile([128, 1], FP32, tag="rstd")
    nc.scalar.activation(out=rstd, in_=lnv, func=F.Exp, scale=-0.5)
    nc.vector.tensor_scalar(out=dst[:, t, :], in0=src[:, t, :],
                            scalar1=mv[:, 0:1], scalar2=rstd,
                            op0=OP.subtract, op1=OP.mult)


def _layernorm(nc, sm, src, dst, eps_t):
    """src [128, NT, 768] fp16 -> dst [128, NT, 768] bf16, per-row LN without
    affine (folded into downstream weights). The Ln/Exp rstd ops are batched
    across all NT tiles (2 ACT ops instead of 2*NT) since this runs inside the
    ACT-bound attention window."""
    mv4 = sm.tile([128, NT, 2], FP32, tag="mv")
    for t in range(NT):
        stats = sm.tile([128, 3, 6], FP32, tag="bn")
        xg = src[:, t, :].rearrange("p (s d) -> p s d", s=3)
        for s in range(3):
            nc.vector.bn_stats(out=stats[:, s, :], in_=xg[:, s, :])
        nc.vector.bn_aggr(out=mv4[:, t, :], in_=stats)
    lnv4 = sm.tile([128, NT], FP32, tag="lnv")
    nc.scalar.activation(out=lnv4, in_=mv4[:, :, 1], func=F.Ln, bias=eps_t)
    rstd4 = sm.tile([128, NT], FP32, tag="rstd")
    nc.scalar.activation(out=rstd4, in_=lnv4, func=F.Exp, scale=-0.5)
    for t in range(NT):
        nc.vector.tensor_scalar(out=dst[:, t, :], in0=src[:, t, :],
                                scalar1=mv4[:, t, 0:1], scalar2=rstd4[:, t:t + 1],
                                op0=OP.subtract, op1=OP.mult)


_CACHE = {}


def _get_module():
    if "nc" not in _CACHE:
        _CACHE["nc"] = _build_module()
    return _CACHE["nc"]


def _pack_f8(a, scale):
    """fp32 array -> fp8e4m3 (TRN variant, clip +-240) -> bf16 byte pairs."""
    q = np.clip(np.asarray(a, np.float32) * scale, -240.0, 240.0).astype(f8e4)
    return np.ascontiguousarray(q).ravel().view(bf16)


def _prep_inputs(inputs):
    x = np.ascontiguousarray(np.asarray(inputs["x"], dtype=np.float32)
                             .astype(np.float16))
    adj = np.asarray(inputs["adj"])
    g1 = np.asarray(inputs["g1"], dtype=np.float32)
    b1 = np.asarray(inputs["b1"], dtype=np.float32)
    g2 = np.asarray(inputs["g2"], dtype=np.float32)
    b2 = np.asarray(inputs["b2"], dtype=np.float32)
    scale = HD ** -0.5

    wq = np.asarray(inputs["wq"], np.float32)
    wk = np.asarray(inputs["wk"], np.float32)
    wv = np.asarray(inputs["wv"], np.float32)
    w1 = np.asarray(inputs["w_ffn1"], np.float32)

    consts = {
        "wq": _pack_f8((g1[:, None] * wq) * scale, SWQ),
        "wk": _pack_f8(g1[:, None] * wk, SW),
        "wv": _pack_f8(g1[:, None] * wv, SW),
        "wo": _pack_f8(np.asarray(inputs["wo"], np.float32), SW),
        "w1": _pack_f8((g2[:, None] * w1).reshape(6, 128, 24, 128)
                       .transpose(2, 1, 0, 3).reshape(24, 128, 768), SW),
        "w2": _pack_f8(np.asarray(inputs["w_ffn2"], np.float32), SW),
        "bq": ((b1 @ wq + np.asarray(inputs["bq"], np.float32)) * scale),
        "bk": (b1 @ wk + np.asarray(inputs["bk"], np.float32)),
        "bv": (b1 @ wv + np.asarray(inputs["bv"], np.float32)).astype(bf16),
        # bo/b2f ride K=1 matmuls into the already-scaled PSUM groups
        "bo": (np.asarray(inputs["bo"], np.float32) * SAO * SW).astype(bf16),
        "b1f": (b2 @ w1 + np.asarray(inputs["b_ffn1"], np.float32)),
        "b2f": (np.asarray(inputs["b_ffn2"], np.float32) * SW).astype(bf16),
    }
    maskT = np.ascontiguousarray(adj.transpose(0, 2, 1)).astype(bf16)
    consts["ident"] = np.eye(128, dtype=np.float32).astype(bf16)
    for nm in ("bq", "bk", "b1f"):
        consts[nm] = consts[nm].astype(bf16)

    # assemble the single blob (see _SPEC)
    wb_const = np.concatenate(
        [np.ascontiguousarray(consts[nm]).ravel()
         for nm, _, _ in _SPEC if nm not in ("maskT4", "x4")])

    in_maps = []
    for c in range(NCORES):
        wb = np.concatenate(
            [wb_const, maskT[c * GPC:(c + 1) * GPC].ravel(),
             x[c * GPC:(c + 1) * GPC].ravel().view(bf16)])
        assert wb.size == _BF16_TOT
        in_maps.append({"wb": wb})
    return in_maps


def _run(inputs, **kwargs):
    nc = _get_module()
    in_maps = _prep_inputs(inputs)
    res = run_bass_kernel_spmd(nc, in_maps, core_ids=list(range(NCORES)), **kwargs)
    out = np.concatenate([res.results[c]["out4"] for c in range(NCORES)],
                         axis=0).astype(np.float32)
    return out, res


def kernel(**inputs) -> np.ndarray:
    out, _ = _run(inputs)
    return out


def run_traced(inputs):
    """For test.py: returns (output, BassKernelResults with profile info)."""
    return _run(inputs, trace=True)


# revision 68
# speedup vs baseline: 1.0083x; 1.0083x over previous
"""DenseGAT layer (LN -> masked 12-head attention -> LN -> GELU FFN, residuals)
on 8 Trainium2 NeuronCores, data-parallel over the batch (4 graphs/core).

Math notes (validated against the reference in fp64/numpy):
- LN affine (g,b) is folded into the following projection weights on the host,
  along with the 1/sqrt(hd) attention scale (into wq) and 0-biases.
- Scores are computed directly transposed, scoresT[m,n] = kT_h^T-slice @ qT_h,
  so softmax's reduction lands on the matmul contraction axis: masked exp
  weights eT are multiplied by the host-transposed 0/1 mask, and the
  denominator comes for free as a 65th "ones" column of V. No max-subtraction
  is needed: |scores| < ~3 for this model family (exp is safe in fp32).
- The projection/FFN matmuls (QKV, O, FFN1, FFN2) run in fp8e4m3 with
  DoubleRow perf mode (2 k-tiles of 128 contracted per pass, ~1.5-2x PE
  throughput); operands carry power-of-2 scales (z x16, weights x64/x512,
  ao x32 via a 1/32 "ones" column) that are divided back out at each PSUM
  evacuation. Empirical max-rel-err of this scheme vs the fp32 reference is
  ~1.8e-2 (gate 2e-2, deterministic inputs). Scores/AV matmuls stay bf16
  (contraction 64 gains nothing from DoubleRow); LN statistics and softmax
  normalization stay fp32; x/out/outres are fp16.
"""
import numpy as np
import ml_dtypes

import concourse.bass as bass
import concourse.mybir as mybir
import concourse.tile as tile
from concourse import masks
from concourse.bass_utils import run_bass_kernel_spmd

bf16 = ml_dtypes.bfloat16
f8e4 = ml_dtypes.float8_e4m3    # TRN fp8_exp4: max normal +-240
FP32 = mybir.dt.float32
FP16 = mybir.dt.float16
BF16 = mybir.dt.bfloat16
F8E4 = mybir.dt.float8e4
F = mybir.ActivationFunctionType
OP = mybir.AluOpType
DR = mybir.MatmulPerfMode.DoubleRow

B, N, D, H = 32, 512, 768, 12
HD = D // H            # 64
NCORES = 8
GPC = B // NCORES      # graphs per core
NT = N // 128          # 4 n-tiles (also m-chunks)
DC = D // 128          # 6 d-chunks
DP = DC // 2           # 3 d-chunk PAIRS (DoubleRow contraction steps)
FC = 4 * D // 128      # 24 ffn chunks
FP = FC // 2           # 12 ffn chunk pairs
EPS = 1e-5

# fp8 operand scales (powers of 2; divided back out at PSUM evacuation)
SZ = 16.0              # z / z2 (LN outputs, unit variance)
SWQ = 512.0            # wq (carries the extra 1/8 attention scale)
SW = 64.0              # wk, wv, wo, w1, w2 (~0.02 scale weights)
SAO = 32.0             # ao (attention out, via 1/SAO ones column in V)


def _split_waits(nc):
    """This walrus accepts one sync-wait per instruction (two for
    EventSemaphore); hoist excess waits onto same-engine nops inserted before
    the instruction (engines dispatch in order, so semantics are preserved)."""
    counter = 0
    for f in nc.m.functions:
        for blk in f.blocks:
            insts = blk.instructions
            i = 0
            while i < len(insts):
                inst = insts[i]
                si = getattr(inst, "sync_info", None)
                waits = list(si.on_wait) if si is not None and si.on_wait else []
                cap = 2 if isinstance(inst, mybir.InstEventSemaphore) else 1
                if len(waits) > cap:
                    si.on_wait = waits[-cap:]
                    for c in waits[:-cap]:
                        counter += 1
                        nop = mybir.InstNoOp(name=f"wsplit-{counter}", ins=[], outs=[])
                        nop.engine = inst.engine
                        nop.sync_info = type(si)(on_wait=[c], on_update=[])
                        insts.insert(i, nop)
                        i += 1
                i += 1


# ALL inputs (weights, biases, per-core mask, and x itself) live in ONE 1-D
# bf16 blob: every PJRT argument costs measurable per-exec dispatch time
# through the tunnel, so the kernel takes a single input tensor. fp8 weights
# are packed two-bytes-per-bf16-element (x4 fp16 likewise) and bitcast on
# chip.
_SPEC = [
    # (name, element count in the STORED dtype, is_fp8_packed)
    ("wq", D * D // 2, True), ("wk", D * D // 2, True),
    ("wv", D * D // 2, True), ("wo", D * D // 2, True),
    ("w1", FC * 128 * D // 2, True), ("w2", 4 * D * D // 2, True),
    ("bv", D, False), ("bo", D, False), ("b2f", D, False),
    ("bq", D, False), ("bk", D, False), ("b1f", 4 * D, False),
    ("ident", 128 * 128, False),     # PE-transpose identity (gpsimd
                                     # affine_select is slow on HW)
    ("maskT4", GPC * N * N, False),  # per-core entries last
    ("x4", GPC * N * D, False),      # fp16 bytes in bf16 slots (bitcast)
]
_BF16_OFF = {}
_off = 0
for _nm, _sz, _f8 in _SPEC:
    _BF16_OFF[_nm] = _off
    _off += _sz
_BF16_TOT = _off


def _build_module():
    nc = bass.Bass()
    dram = {
        "wb": nc.dram_tensor("wb", [_BF16_TOT], BF16, kind="ExternalInput"),
    }
    out4 = nc.dram_tensor("out4", [GPC, N, D], FP16, kind="ExternalOutput")

    with tile.TileContext(nc) as tc:
        _emit(nc, tc, dram, out4)
    _split_waits(nc)
    return nc


def _wb(dram, name, sz, extra_off=0):
    return dram["wb"][_BF16_OFF[name] + extra_off:
                      _BF16_OFF[name] + extra_off + sz]


def _emit(nc, tc, dram, out4):
    import contextlib
    ctx = contextlib.ExitStack()
    with ctx:
        res = ctx.enter_context(tc.tile_pool(name="res", bufs=1))       # resident
        xp = ctx.enter_context(tc.tile_pool(name="xp", bufs=2))
        mkp = ctx.enter_context(tc.tile_pool(name="mkp", bufs=1))
        znat = ctx.enter_context(tc.tile_pool(name="znat", bufs=2))     # z / ao / z2
        ztp = ctx.enter_context(tc.tile_pool(name="ztp", bufs=2))       # zT / aoT / z2T (bf16 staging)
        zt8p = ctx.enter_context(tc.tile_pool(name="zt8p", bufs=2))     # fp8 transposed acts
        qkp = ctx.enter_context(tc.tile_pool(name="qkp", bufs=1))       # qT, kT
        vp = ctx.enter_context(tc.tile_pool(name="vp", bufs=1))
        etp = ctx.enter_context(tc.tile_pool(name="etp", bufs=7))
        gp = ctx.enter_context(tc.tile_pool(name="gp", bufs=1))         # gT resident per graph
        orp = ctx.enter_context(tc.tile_pool(name="orp", bufs=1))       # outres
        fin = ctx.enter_context(tc.tile_pool(name="fin", bufs=2))
        sm = ctx.enter_context(tc.tile_pool(name="sm", bufs=4))         # small stats tiles
        mm = ctx.enter_context(tc.tile_pool(name="mm", bufs=5, space="PSUM"))
        av = ctx.enter_context(tc.tile_pool(name="av", bufs=3, space="PSUM"))

        # ---- input prefetch (graph 0 first so weight loads don't delay it) --
        x_tiles, mk_tiles = {}, {}
        zT8s, qTs, kTs, vsbs = {}, {}, {}, {}

        def xsrc(b):
            return _wb(dram, "x4", N * D, extra_off=b * N * D).bitcast(
                FP16).rearrange("(t p d) -> p t d", p=128, d=D)

        def load_x(b):
            x_tiles[b] = xp.tile([128, NT, D], FP16, name=f"x{b}", tag="x")
            nc.sync.dma_start(out=x_tiles[b], in_=xsrc(b))

        def load_mask(b):
            mk_tiles[b] = mkp.tile([128, NT, N], BF16, name=f"mk{b}", tag="mk")
            nc.sync.dma_start(
                out=mk_tiles[b],
                in_=_wb(dram, "maskT4", N * N, extra_off=b * N * N)
                .rearrange("(c p n) -> p c n", p=128, n=N))

        # ---- one-time loads -------------------------------------------------
        def load_w8(name, chunks, cols8):
            """fp8 weight packed as bf16 byte pairs: load bf16 [128, chunks,
            cols8/2], return the fp8 [128, chunks, cols8] bitcast view."""
            t = res.tile([128, chunks, cols8 // 2], BF16, tag=name)
            nc.sync.dma_start(
                out=t, in_=_wb(dram, name, chunks * 128 * cols8 // 2)
                .rearrange("(c p d) -> p c d", p=128, d=cols8 // 2))
            return t.bitcast(F8E4)

        # startup: the serial DMA pipe is the bottleneck, so order it by need:
        # all 4 x tiles, then wq's first column-block (enough for qk j=0..2),
        # then the rest. Per-tile LN runs as each x tile lands; graph 0's zT
        # rides the PE (idle at startup) straight into fp8 via the ACT evac.
        eps_t = res.tile([128, 1], FP32, tag="eps")
        nc.vector.memset(eps_t, EPS)
        x_tiles[0] = xp.tile([128, NT, D], FP16, name="x0", tag="x")
        x0src = xsrc(0)
        z0_sb = znat.tile([128, NT, D], BF16, tag="znat", name="z0")
        zT0 = zt8p.tile([128, DC, N], F8E4, tag="zt8", name="zT0")
        for t_ in range(NT):
            nc.sync.dma_start(out=x_tiles[0][:, t_, :], in_=x0src[:, t_, :])
        # identity right behind x0 (32KB): the first zT0 transpose needs it
        ident = res.tile([128, 128], BF16, tag="ident")
        nc.sync.dma_start(out=ident, in_=_wb(dram, "ident", 128 * 128)
                          .rearrange("(p d) -> p d", p=128))
        # wq first column-block first (fp8 cols j<3 of every dc chunk, enough
        # for qk j=0..2), then the rest after the LN chain is queued
        wq_t = res.tile([128, DC, D // 2], BF16, tag="wq")
        wq_src = _wb(dram, "wq", DC * 128 * D // 2).rearrange(
            "(c p d) -> p c d", p=128, d=D // 2)
        nc.sync.dma_start(out=wq_t[:, :, 0:64], in_=wq_src[:, :, 0:64])
        nc.sync.dma_start(out=wq_t[:, :, 64:192], in_=wq_src[:, :, 64:192])
        wq8 = wq_t.bitcast(F8E4)
        # graph 0's zT via PE transposes: the PE is idle at startup while the
        # serial DMA pipe (x, wq, wk...) is the scarce resource. The ACT evac
        # applies the fp8 z-scale (x16) for free.
        for t_ in range(NT):
            _layernorm_tile(nc, sm, x_tiles[0], z0_sb, eps_t, t_)
            for dcb in range(DC):
                pst = mm.tile([128, 128], BF16, tag="mm")
                nc.tensor.matmul(pst[:, :],
                                 z0_sb[:, t_, dcb * 128:(dcb + 1) * 128],
                                 ident[:, :], is_transpose=True)
                nc.scalar.activation(
                    out=zT0[:, dcb, t_ * 128:(t_ + 1) * 128], in_=pst[:, :],
                    func=F.Identity, scale=SZ)
        zT8s[0] = zT0
        nc.sync.dma_start(out=wq_t[:, :, 192:384], in_=wq_src[:, :, 192:384])
        wk8 = load_w8("wk", DC, D)
        wv8 = load_w8("wv", DC, D)
        load_mask(0)

        bq_sb = res.tile([128, DC], FP32, tag="bq")
        bq_lo = res.tile([128, DC], BF16, tag="bqlo")
        nc.sync.dma_start(out=bq_lo,
                          in_=_wb(dram, "bq", D).rearrange("(c p) -> p c", p=128))
        nc.scalar.activation(out=bq_sb, in_=bq_lo, func=F.Identity)
        bk_sb = res.tile([128, DC], FP32, tag="bk")
        bk_lo = res.tile([128, DC], BF16, tag="bklo")
        nc.sync.dma_start(out=bk_lo,
                          in_=_wb(dram, "bk", D).rearrange("(c p) -> p c", p=128))
        nc.scalar.activation(out=bk_sb, in_=bk_lo, func=F.Identity)
        b1f_sb = res.tile([128, FC], BF16, tag="b1f")
        nc.sync.dma_start(out=b1f_sb,
                          in_=_wb(dram, "b1f", 4 * D).rearrange("(c p) -> p c", p=128))

        def bcast(name):
            t = res.tile([128, D], BF16, tag=name + "b")
            src = _wb(dram, name, D)
            nc.sync.dma_start(out=t, in_=bass.AP(
                tensor=src.tensor, offset=src.offset, ap=[[0, 128]] + list(src.ap)))
            return t

        bv_b = bcast("bv")
        bo_b = bcast("bo")      # host-scaled by SAO*SW (O-proj psum scale)
        b2_b = bcast("b2f")     # host-scaled by SW (FFN2 psum scale)
        # all-ones [1, 128] stationary: a K=1 matmul appended to a PSUM
        # accumulation group adds ones^T @ bias_row = the bias broadcast over
        # all 128 output partitions, for free on the PE
        ones1 = res.tile([1, 128], BF16, tag="ones1")
        nc.vector.memset(ones1, 1.0)

        # ---- graph-level software pipeline ---------------------------------
        # LN1/zT and QKV for graph b+1 are emitted inside graph b's body so
        # each engine's static instruction order interleaves next-graph work
        # into this graph's stall windows.

        zTbs = {}

        def ln_z_zT(b):
            z_sb = znat.tile([128, NT, D], BF16, tag="znat", name=f"z{b}")
            zTb = ztp.tile([128, DC, N], BF16, tag="zt", name=f"zTb{b}")
            zT8 = zt8p.tile([128, DC, N], F8E4, tag="zt8", name=f"zT8{b}")
            _layernorm(nc, sm, x_tiles[b], z_sb, eps_t)
            for t in range(NT):
                nc.sync.dma_start_transpose(
                    out=zTb[:, :, t * 128:(t + 1) * 128], in_=z_sb[:, t, :])
            zTbs[b] = zTb
            zT8s[b] = zT8

        def zT_convert(b):
            # emitted a few attention pairs after ln_z_zT so the transposes
            # are long done and the converts neither head-of-line-block the
            # DVE FIFO nor finish after the next graph's qk_proj needs them
            zTb, zT8 = zTbs.pop(b), zT8s[b]
            for t in range(NT):
                nc.vector.tensor_scalar(
                    out=zT8[:, :, t * 128:(t + 1) * 128],
                    in0=zTb[:, :, t * 128:(t + 1) * 128],
                    scalar1=SZ, scalar2=None, op0=OP.mult)

        def qk_proj(b, split_n=False):
            zT8 = zT8s[b]
            qT = qkp.tile([128, DC, N], BF16, tag="q", name=f"qT{b}")
            kT = qkp.tile([128, DC, N], BF16, tag="k", name=f"kT{b}")
            nsplits = ((0, 256), (256, 512)) if split_n else ((0, 512),)
            for wsb, bsb, inv, outT, on_act in (
                    (wq8, bq_sb, 1.0 / (SZ * SWQ), qT, True),
                    (wk8, bk_sb, 1.0 / (SZ * SW), kT, True)):
                for j in range(DC):
                    ps = mm.tile([128, N], FP32, tag="mm")
                    for n0, n1 in nsplits:
                        for dp in range(DP):
                            nc.tensor.matmul(
                                ps[:, n0:n1],
                                wsb[:, 2 * dp:2 * dp + 2, j * 128:(j + 1) * 128],
                                zT8[:, 2 * dp:2 * dp + 2, n0:n1],
                                start=(dp == 0), stop=(dp == DP - 1),
                                perf_mode=DR)
                    # evacs split across ACT (q) and DVE (k) to balance the
                    # two engines' loads in the post-attention window
                    if on_act:
                        nc.scalar.activation(
                            out=outT[:, j, :], in_=ps[:, :], func=F.Identity,
                            scale=inv, bias=bsb[:, j:j + 1])
                    else:
                        nc.vector.tensor_scalar(
                            out=outT[:, j, :], in0=ps[:, :],
                            scalar1=inv, scalar2=bsb[:, j:j + 1],
                            op0=OP.mult, op1=OP.add)
            qTs[b], kTs[b] = qT, kT

        def v_proj(b):
            zT8 = zT8s[b]
            v_sb = vp.tile([128, NT, H, HD + 1], BF16, name=f"v{b}", tag="v")
            for mc in range(NT):
                for half in range(2):
                    # psum from the av pool: it is idle in this window (pav
                    # tiles are dead post-attention) while the mm ring's WAR
                    # against the O-proj evacs would stall these matmuls
                    ps = av.tile([128, 384], FP32, tag="av")
                    for dp in range(DP):
                        nc.tensor.matmul(
                            ps[:, :],
                            zT8[:, 2 * dp:2 * dp + 2, mc * 128:(mc + 1) * 128],
                            wv8[:, 2 * dp:2 * dp + 2, half * 384:(half + 1) * 384],
                            start=(dp == 0), stop=(dp == DP - 1),
                            perf_mode=DR)
                    nc.vector.scalar_tensor_tensor(
                        out=v_sb[:, mc, half * 6:(half + 1) * 6, 0:HD],
                        in0=ps.rearrange("p (h d) -> p h d", d=HD),
                        scalar=1.0 / (SZ * SW),
                        in1=bv_b[:, half * 384:(half + 1) * 384].rearrange(
                            "p (h d) -> p h d", d=HD),
                        op0=OP.mult, op1=OP.add)
            # ones column carries 1/SAO so the evac reciprocal bakes the fp8
            # ao scale into the normalized output for free
            nc.vector.memset(v_sb[:, :, :, HD:HD + 1], 1.0 / SAO)
            vsbs[b] = v_sb

        eTs_pre = {}

        def scores_pair(p, qT, kT, mk_sb, gtag):
            eT = etp.tile([128, NT, 2, N], BF16, tag="et", name=f"eT{gtag}_{p}")
            for mc in range(NT):
                for sub in range(2):
                    off = sub * 64
                    ps = mm.tile([128, N], FP32, tag="mm")
                    nc.tensor.matmul(
                        ps[:, :],
                        kT[off:off + 64, p, mc * 128:(mc + 1) * 128],
                        qT[off:off + 64, p, :],
                        start=True, stop=True)
                    nc.scalar.activation(out=eT[:, mc, sub, :], in_=ps[:, :],
                                         func=F.Exp)
                # one masked-multiply for both sub-heads (mask broadcast over
                # the sub dim) — halves the DVE op count in this hot window
                mk1 = mk_sb[:, mc, :]
                mkb = bass.AP(tensor=mk1.tensor, offset=mk1.offset,
                              ap=[mk1.ap[0], [0, 2], mk1.ap[1]])
                nc.vector.tensor_tensor(out=eT[:, mc, :, :],
                                        in0=eT[:, mc, :, :],
                                        in1=mkb, op=OP.mult)
            return eT

        qk_proj(0, split_n=True)
        v_proj(0)
        # x1 isn't needed until LN1(1) during graph 0's attention (~30us);
        # keep its transfer out of the startup pipe
        with tc.tile_wait_until(0.012):
            load_x(1)
        # wo/w1/w2 aren't needed until O-proj/FFN of graph 0 (~60-90us in);
        # keep their transfers out of the startup DMA-pipe rush
        with tc.tile_wait_until(0.018):
            wo8 = load_w8("wo", DC, D)
            w2_8 = load_w8("w2", FC, D)
        with tc.tile_wait_until(0.025):
            w1t = res.tile([128, FC, D // 2], BF16, tag="w1")
            nc.gpsimd.dma_start(
                out=w1t, in_=_wb(dram, "w1", FC * 128 * D // 2)
                .rearrange("(f p d) -> p f d", p=128, d=D // 2))
            w1_8 = w1t.bitcast(F8E4)
        w1v = w1_8.rearrange("p f (c m) -> p f c m", m=128)

        for b in range(GPC):
            x_sb = x_tiles[b]
            mk_sb = mk_tiles.pop(b)
            qT, kT, v_sb = qTs.pop(b), kTs.pop(b), vsbs.pop(b)

            # ---- attention: 6 head-pairs, software-pipelined ----
            ao = znat.tile([128, NT, D], BF16, tag="znat", name=f"ao{b}")
            pav = {}

            def scores_block(p):
                return scores_pair(p, qT, kT, mk_sb, b)

            def av_block(p, eT):
                half = p // 3
                if half not in pav:
                    pav[half] = [av.tile([128, 6 * (HD + 1)], FP32, tag="av",
                                         name=f"pav{half}_{nc4i}")
                                 for nc4i in range(NT)]
                for sub in range(2):
                    h = 2 * p + sub
                    lane = h - half * 6
                    for nc4 in range(NT):
                        for mc in range(NT):
                            nc.tensor.matmul(
                                pav[half][nc4][:, lane * 65:lane * 65 + 65],
                                eT[:, mc, sub, nc4 * 128:(nc4 + 1) * 128],
                                v_sb[:, mc, h, :],
                                start=(mc == 0), stop=(mc == NT - 1))

            def evac_block(half):
                for nc4 in range(NT):
                    rec = sm.tile([128, 6], FP32, tag="rec")
                    nc.vector.reciprocal(out=rec, in_=pav[half][nc4][:, HD::HD + 1])
                    rb = bass.AP(tensor=rec.tensor, offset=rec.offset,
                                 ap=[rec.ap[0], [rec.ap[1][0], 6], [0, HD]])
                    nc.vector.tensor_tensor(
                        out=ao[:, nc4, half * 384:(half + 1) * 384].rearrange(
                            "p (h d) -> p h d", d=HD),
                        in0=pav[half][nc4].rearrange(
                            "p (h s) -> p h s", s=HD + 1)[:, :, 0:HD],
                        in1=rb, op=OP.mult)

            pre = eTs_pre.pop(b, None)
            if pre:
                queue = list(pre)
                nxt = len(pre)
            else:
                queue = [scores_block(0)]
                nxt = 1
            for p in range(6):
                if nxt < 6:
                    queue.append(scores_block(nxt))
                    nxt += 1
                eT_cur = queue.pop(0)
                av_block(p, eT_cur)
                if p == 0 and b + 1 < GPC:
                    # next graph's LN1+transpose: its DVE/ACT/DMA work fills
                    # this graph's attention slack, and zT(b+1) is ready before
                    # the post-attention QK(b+1) matmuls need it.
                    ln_z_zT(b + 1)
                if p == 3 and b + 1 < GPC:
                    zT_convert(b + 1)
                if p == 2:
                    evac_block(0)
                elif p == 5:
                    evac_block(1)
                    if b + 1 < GPC:
                        load_mask(b + 1)

            # aoT (bf16 staging transpose, per-tile fp8 convert rides behind)
            aoTb = ztp.tile([128, DC, N], BF16, tag="zt", name=f"aoTb{b}")
            aoT8 = zt8p.tile([128, DC, N], F8E4, tag="zt8", name=f"aoT8{b}")
            for t in range(NT):
                nc.sync.dma_start_transpose(out=aoTb[:, :, t * 128:(t + 1) * 128],
                                            in_=ao[:, t, :])
                nc.vector.tensor_scalar(
                    out=aoT8[:, :, t * 128:(t + 1) * 128],
                    in0=aoTb[:, :, t * 128:(t + 1) * 128],
                    scalar1=1.0, scalar2=None, op0=OP.mult)

            # next graph's q/k projections fill the aoT-transpose latency
            if b + 1 < GPC:
                qk_proj(b + 1)

            # O projection + residual -> outres (fp16), LN2 interleaved per
            # tile so z2T(t) transposes overlap O-proj of tiles t+1..3 (the
            # DVE/ACT LN2 work for tile t queues before tile t+1's evac).
            outres = orp.tile([128, NT, D], FP16, name=f"or{b}", tag="or")
            z2 = znat.tile([128, NT, D], BF16, tag="znat", name=f"z2{b}")
            z2Tb = ztp.tile([128, DC, N], BF16, tag="zt", name=f"z2Tb{b}")
            z2T8 = zt8p.tile([128, DC, N], F8E4, tag="zt8", name=f"z2T8{b}")
            for nc4 in range(NT):
                for half in range(2):
                    ps = mm.tile([128, 384], FP32, tag="mm")
                    for dp in range(DP):
                        nc.tensor.matmul(
                            ps[:, :],
                            aoT8[:, 2 * dp:2 * dp + 2, nc4 * 128:(nc4 + 1) * 128],
                            wo8[:, 2 * dp:2 * dp + 2, half * 384:(half + 1) * 384],
                            start=(dp == 0), stop=False,
                            perf_mode=DR)
                    # + bo (host-scaled) via a K=1 ones-row matmul in-group
                    nc.tensor.matmul(
                        ps[:, :], ones1[:, :],
                        bo_b[0:1, half * 384:(half + 1) * 384],
                        start=False, stop=True)
                    # evacuation (descale) + residual
                    nc.vector.scalar_tensor_tensor(
                        out=outres[:, nc4, half * 384:(half + 1) * 384],
                        in0=ps[:, :], scalar=1.0 / (SAO * SW),
                        in1=x_sb[:, nc4, half * 384:(half + 1) * 384],
                        op0=OP.mult, op1=OP.add)
                # LN2(tile nc4) via DVE bn_stats (keeps ACT free for the
                # next-graph exp precompute below)
                t = nc4
                _layernorm_tile(nc, sm, outres, z2, eps_t, t)
                # ACT queue: fires right after this tile's rstd instead of
                # queuing behind unrelated SP configs
                nc.scalar.dma_start_transpose(
                    out=z2Tb[:, :, t * 128:(t + 1) * 128], in_=z2[:, t, :])

            # next graph's v-proj + first scores pair: their PE/ACT work fills
            # the LN2 -> z2T transpose -> fp8 convert latency before FFN1.
            # The z2T8 converts are emitted after v_proj so the DVE FIFO
            # reaches the O-proj evacs (v_proj's mm-ring WAR blocker) sooner.
            # converts first: with v_proj WAR-free on the av pool, its DVE
            # evacs would only delay the z2T8 converts FFN1 waits on
            for t in range(NT):
                nc.vector.tensor_scalar(
                    out=z2T8[:, :, t * 128:(t + 1) * 128],
                    in0=z2Tb[:, :, t * 128:(t + 1) * 128],
                    scalar1=SZ, scalar2=None, op0=OP.mult)
            if b + 1 < GPC:
                v_proj(b + 1)
            if b + 1 < GPC:
                eTs_pre.setdefault(b + 1, []).append(
                    scores_pair(0, qTs[b + 1], kTs[b + 1], mk_tiles[b + 1],
                                b + 1))

            # FFN1 + gelu -> gT (fp8). First chunks run as n-splits so they
            # only need the early z2T transposes (later ones still in flight
            # after O-proj).
            gT8 = gp.tile([128, FC, N], F8E4, name=f"gT{b}", tag="gT")
            for fc in range(FC):
                ps = mm.tile([128, N], FP32, tag="mm")
                if fc < 2:        # quarters: starts after z2T transpose t=0
                    nsp = ((0, 128), (128, 256), (256, 384), (384, 512))
                elif fc < 4:      # halves: needs t=0,1
                    nsp = ((0, 256), (256, 512))
                else:
                    nsp = ((0, 512),)
                for n0, n1 in nsp:
                    for dp in range(DP):
                        nc.tensor.matmul(
                            ps[:, n0:n1],
                            w1v[:, fc, 2 * dp:2 * dp + 2, :],
                            z2T8[:, 2 * dp:2 * dp + 2, n0:n1],
                            start=(dp == 0), stop=(dp == DP - 1),
                            perf_mode=DR)
                # NOTE: no exp ops may interleave here — gelu and exp live in
                # different ACT table sets, and each switch costs a ~2.7us
                # ACT_TABLE_LOAD (invisible to TimelineSim)
                nc.scalar.activation(out=gT8[:, fc, :], in_=ps[:, :], func=F.Gelu,
                                     bias=b1f_sb[:, fc:fc + 1],
                                     scale=1.0 / (SZ * SW))

            if b + 2 < GPC:
                load_x(b + 2)

            # FFN2 + final residual -> out. ACT is idle here, so pre-compute
            # two more of the next graph's score pairs (exp on ACT) — the
            # next attention window then starts with 4/6 pairs done.
            for nc4 in range(NT):
                for half in range(2):
                    ps = mm.tile([128, 384], FP32, tag="mm")
                    for fp in range(FP):
                        nc.tensor.matmul(
                            ps[:, :],
                            gT8[:, 2 * fp:2 * fp + 2, nc4 * 128:(nc4 + 1) * 128],
                            w2_8[:, 2 * fp:2 * fp + 2, half * 384:(half + 1) * 384],
                            start=(fp == 0), stop=False,
                            perf_mode=DR)
                    # + b2f (host-scaled) in-group, as with bo above
                    nc.tensor.matmul(
                        ps[:, :], ones1[:, :],
                        b2_b[0:1, half * 384:(half + 1) * 384],
                        start=False, stop=True)
                    ft = fin.tile([128, 384], FP16, tag="fin")
                    nc.vector.scalar_tensor_tensor(
                        out=ft, in0=ps[:, :], scalar=1.0 / SW,
                        in1=outres[:, nc4, half * 384:(half + 1) * 384],
                        op0=OP.mult, op1=OP.add)
                    nc.sync.dma_start(
                        out=out4[b, nc4 * 128:(nc4 + 1) * 128,
                                 half * 384:(half + 1) * 384],
                        in_=ft)
                # pre-compute next graph's score pairs here: the FFN2 window
                # has no ACT work of its own, and exp shares the loaded ACT
                # table set (the gelu run above already paid its two loads)
                if b + 1 < GPC and nc4 < 3:
                    eTs_pre[b + 1].append(
                        scores_pair(nc4 + 1, qTs[b + 1], kTs[b + 1],
                                    mk_tiles[b + 1], b + 1))
            del x_tiles[b]


def _layernorm_tile(nc, sm, src, dst, eps_t, t):
    """Single-tile LN chain (stats -> rstd -> z) for pipeline fill: no
    cross-tile batching, so tile t's output is ready as soon as tile t's
    input is."""
    stats = sm.tile([128, 3, 6], FP32, tag="bn")
    xg = src[:, t, :].rearrange("p (s d) -> p s d", s=3)
    for s in range(3):
        nc.vector.bn_stats(out=stats[:, s, :], in_=xg[:, s, :])
    mv = sm.tile([128, 2], FP32, tag="mv")
    nc.vector.bn_aggr(out=mv, in_=stats)
    lnv = sm.tile([128, 1], FP32, tag="lnv")
    nc.scalar.activation(out=lnv, in_=mv[:, 1:2], func=F.Ln, bias=eps_t)
    rstd = sm.tile([128, 1], FP32, tag="rstd")
    nc.scalar.activation(out=rstd, in_=lnv, func=F.Exp, scale=-0.5)
    nc.vector.tensor_scalar(out=dst[:, t, :], in0=src[:, t, :],
                            scalar1=mv[:, 0:1], scalar2=rstd,
                            op0=OP.subtract, op1=OP.mult)


def _layernorm(nc, sm, src, dst, eps_t):
    """src [128, NT, 768] fp16 -> dst [128, NT, 768] bf16, per-row LN without
    affine (folded into downstream weights). The Ln/Exp rstd ops are batched
    across all NT tiles (2 ACT ops instead of 2*NT) since this runs inside the
    ACT-bound attention window."""
    mv4 = sm.tile([128, NT, 2], FP32, tag="mv")
    for t in range(NT):
        stats = sm.tile([128, 3, 6], FP32, tag="bn")
        xg = src[:, t, :].rearrange("p (s d) -> p s d", s=3)
        for s in range(3):
            nc.vector.bn_stats(out=stats[:, s, :], in_=xg[:, s, :])
        nc.vector.bn_aggr(out=mv4[:, t, :], in_=stats)
    lnv4 = sm.tile([128, NT], FP32, tag="lnv")
    nc.scalar.activation(out=lnv4, in_=mv4[:, :, 1], func=F.Ln, bias=eps_t)
    rstd4 = sm.tile([128, NT], FP32, tag="rstd")
    nc.scalar.activation(out=rstd4, in_=lnv4, func=F.Exp, scale=-0.5)
    for t in range(NT):
        nc.vector.tensor_scalar(out=dst[:, t, :], in0=src[:, t, :],
                                scalar1=mv4[:, t, 0:1], scalar2=rstd4[:, t:t + 1],
                                op0=OP.subtract, op1=OP.mult)


_CACHE = {}


def _get_module():
    if "nc" not in _CACHE:
        _CACHE["nc"] = _build_module()
    return _CACHE["nc"]


def _pack_f8(a, scale):
    """fp32 array -> fp8e4m3 (TRN variant, clip +-240) -> bf16 byte pairs."""
    q = np.clip(np.asarray(a, np.float32) * scale, -240.0, 240.0).astype(f8e4)
    return np.ascontiguousarray(q).ravel().view(bf16)


def _prep_inputs(inputs):
    x = np.ascontiguousarray(np.asarray(inputs["x"], dtype=np.float32)
                             .astype(np.float16))
    adj = np.asarray(inputs["adj"])
    g1 = np.asarray(inputs["g1"], dtype=np.float32)
    b1 = np.asarray(inputs["b1"], dtype=np.float32)
    g2 = np.asarray(inputs["g2"], dtype=np.float32)
    b2 = np.asarray(inputs["b2"], dtype=np.float32)
    scale = HD ** -0.5

    wq = np.asarray(inputs["wq"], np.float32)
    wk = np.asarray(inputs["wk"], np.float32)
    wv = np.asarray(inputs["wv"], np.float32)
    w1 = np.asarray(inputs["w_ffn1"], np.float32)

    consts = {
        "wq": _pack_f8((g1[:, None] * wq) * scale, SWQ),
        "wk": _pack_f8(g1[:, None] * wk, SW),
        "wv": _pack_f8(g1[:, None] * wv, SW),
        "wo": _pack_f8(np.asarray(inputs["wo"], np.float32), SW),
        "w1": _pack_f8((g2[:, None] * w1).reshape(6, 128, 24, 128)
                       .transpose(2, 1, 0, 3).reshape(24, 128, 768), SW),
        "w2": _pack_f8(np.asarray(inputs["w_ffn2"], np.float32), SW),
        "bq": ((b1 @ wq + np.asarray(inputs["bq"], np.float32)) * scale),
        "bk": (b1 @ wk + np.asarray(inputs["bk"], np.float32)),
        "bv": (b1 @ wv + np.asarray(inputs["bv"], np.float32)).astype(bf16),
        # bo/b2f ride K=1 matmuls into the already-scaled PSUM groups
        "bo": (np.asarray(inputs["bo"], np.float32) * SAO * SW).astype(bf16),
        "b1f": (b2 @ w1 + np.asarray(inputs["b_ffn1"], np.float32)),
        "b2f": (np.asarray(inputs["b_ffn2"], np.float32) * SW).astype(bf16),
    }
    maskT = np.ascontiguousarray(adj.transpose(0, 2, 1)).astype(bf16)
    consts["ident"] = np.eye(128, dtype=np.float32).astype(bf16)
    for nm in ("bq", "bk", "b1f"):
        consts[nm] = consts[nm].astype(bf16)

    # assemble the single blob (see _SPEC)
    wb_const = np.concatenate(
        [np.ascontiguousarray(consts[nm]).ravel()
         for nm, _, _ in _SPEC if nm not in ("maskT4", "x4")])

    in_maps = []
    for c in range(NCORES):
        wb = np.concatenate(
            [wb_const, maskT[c * GPC:(c + 1) * GPC].ravel(),
             x[c * GPC:(c + 1) * GPC].ravel().view(bf16)])
        assert wb.size == _BF16_TOT
        in_maps.append({"wb": wb})
    return in_maps


def _run(inputs, **kwargs):
    nc = _get_module()
    in_maps = _prep_inputs(inputs)
    res = run_bass_kernel_spmd(nc, in_maps, core_ids=list(range(NCORES)), **kwargs)
    out = np.concatenate([res.results[c]["out4"] for c in range(NCORES)],
                         axis=0).astype(np.float32)
    return out, res


def kernel(**inputs) -> np.ndarray:
    out, _ = _run(inputs)
    return out


def run_traced(inputs):
    """For test.py: returns (output, BassKernelResults with profile info)."""
    return _run(inputs, trace=True)
